# revision 30
# baseline (speedup 1.0000x reference)
"""Multi-head self-attention (B=2, S=2048, D=1024, H=16, causal) on 8 NeuronCores.

Sharding: core c = 4*b + g handles batch b and heads 4g..4g+3 (batch x
head-group parallel). Per core:
  - q/k projections in transposed layout  qT/kT [dh, s]  (dh on partitions)
  - v projection in natural layout [s, dh] with a fused ones-column per head
    (gives the softmax denominator for free during the AV matmul)
  - causal attention in scoresT [j, i] orientation: PE scores -> ACT exp
    (scale=1/8, no max subtraction; scores ~ N(0,1) so exp never overflows)
    -> DVE causal mask multiply on diagonal blocks -> PE AV accumulate
  - normalization of attnT by the per-query denominator via a PE ones-
    broadcast + DVE multiply during PSUM eviction
  - partial o-projection out_c = merged_c @ Wo[:, cols_c].T
Host sums the 4 partial outputs per batch (the only cross-core reduction).

All matmuls run in float32r (full-rate fp32 on the TRN2 PE).
"""

import numpy as np

import concourse.bass as bass
from concourse import bacc
import concourse.mybir as mybir
import concourse.tile as tile
from concourse import bass_utils

F32 = mybir.dt.float32
F32R = mybir.dt.float32r
EXP = mybir.ActivationFunctionType.Exp

B, S, D = 2, 2048, 1024
H, DH = 16, 64
NCORES = 8
HPG = 4                  # heads per group (per core)
M = HPG * DH             # 256 per-core head dims
DC = D // 128            # 8 contraction chunks for projections
IC = 512                 # i (query) chunk for attention
JC = 128                 # j (key) chunk for attention
SCALE = 1.0 / np.sqrt(DH)


def _build_nc():
    nc = bacc.Bacc("TRN2", target_bir_lowering=False, debug=False)

    xT_d = nc.dram_tensor("xT", [D, S], F32R, kind="ExternalInput").ap()
    wqkv_d = nc.dram_tensor("wqkvT", [D, 3 * M], F32R, kind="ExternalInput").ap()
    woT_d = nc.dram_tensor("woT", [M, D], F32R, kind="ExternalInput").ap()
    mask_d = nc.dram_tensor("mask", [JC, 896], F32, kind="ExternalInput").ap()
    onesa_d = nc.dram_tensor("ones_a", [1, 64], F32R, kind="ExternalInput").ap()
    onesb_d = nc.dram_tensor("ones_b", [JC, HPG], F32R, kind="ExternalInput").ap()
    out_d = nc.dram_tensor("out", [S, D], F32, kind="ExternalOutput").ap()

    with tile.TileContext(nc) as tc:
        _body(tc, xT_d, wqkv_d, woT_d, mask_d, onesa_d, onesb_d, out_d)
    nc.compile()
    return nc


def _body(tc, xT_d, wqkv_d, woT_d, mask_d, onesa_d, onesb_d, out_d):
    nc = tc.nc
    from contextlib import ExitStack
    ctx = ExitStack()
    with ctx:
        p_x = ctx.enter_context(tc.tile_pool(name="x", bufs=DC))
        p_w = ctx.enter_context(tc.tile_pool(name="w", bufs=DC))
        p_wo = ctx.enter_context(tc.tile_pool(name="wo", bufs=2))
        p_qk = ctx.enter_context(tc.tile_pool(name="qk", bufs=2))
        p_v = ctx.enter_context(tc.tile_pool(name="v", bufs=S // JC))
        p_mg = ctx.enter_context(tc.tile_pool(name="mg", bufs=2))
        p_probs = ctx.enter_context(tc.tile_pool(name="probs", bufs=5))
        p_small = ctx.enter_context(tc.tile_pool(name="small", bufs=1))
        p_mask = ctx.enter_context(tc.tile_pool(name="mask", bufs=1))
        p_ostg = ctx.enter_context(tc.tile_pool(name="ostg", bufs=2))
        p_ones = ctx.enter_context(tc.tile_pool(name="ones", bufs=1))

        ps_big = ctx.enter_context(tc.tile_pool(name="psb", bufs=2, space="PSUM"))
        ps_sc = ctx.enter_context(tc.tile_pool(name="pss", bufs=2, space="PSUM"))
        ps_at = ctx.enter_context(tc.tile_pool(name="psa", bufs=2, space="PSUM"))

        # ---- input loads: combined qkv weight tile + x tile interleaved in
        # the order the projection matmuls consume them (dc ascending), so
        # the first matmul can start ~5us in instead of after the full load.
        w_t, x_t = [], []
        for dc in range(DC):
            wt = p_w.tile([128, 3 * M], F32R, tag="w")
            nc.sync.dma_start(wt[:], wqkv_d[dc * 128:(dc + 1) * 128, :])
            w_t.append(wt)
            xt = p_x.tile([128, S], F32R, tag="x")
            for xc in range(4):  # 512-col chunks: s4-block matmuls gate on
                nc.sync.dma_start(  # their own chunk, not the whole tile
                    xt[:, xc * 512:(xc + 1) * 512],
                    xT_d[dc * 128:(dc + 1) * 128, xc * 512:(xc + 1) * 512])
            x_t.append(xt)
        wo_t = []
        for kc in range(2):
            t = p_wo.tile([128, D], F32R, tag="wo")
            nc.sync.dma_start(t[:], woT_d[kc * 128:(kc + 1) * 128, :])
            wo_t.append(t)
        mask_t = p_mask.tile([JC, 896], F32, tag="mask")
        nc.sync.dma_start(mask_t[:], mask_d[:])
        ones_t = p_ones.tile([1, 64], F32R, tag="ones")
        nc.sync.dma_start(ones_t[:], onesa_d[:])
        onesb_t = p_ones.tile([JC, HPG], F32R, tag="onesb")
        nc.sync.dma_start(onesb_t[:], onesb_d[:])

        # ---- projection building blocks ----
        q_t, k_t = {}, {}

        def qk_proj(mc):
            # qT/kT [m, s] = sum_d WT[d, m] xT[d, s], m-chunk mc
            for woff, store, tg in ((0, q_t, "qT"), (M, k_t, "kT")):
                dst = p_qk.tile([128, S], F32R, tag=tg, name=f"{tg}{mc}")
                for s4 in range(S // 512):
                    ps = ps_big.tile([128, 512], F32, tag="proj")
                    for dc in range(DC):
                        nc.tensor.matmul(
                            ps[:],
                            w_t[dc][:, woff + mc * 128:woff + (mc + 1) * 128],
                            x_t[dc][:, s4 * 512:(s4 + 1) * 512],
                            start=(dc == 0), stop=(dc == DC - 1))
                    nc.scalar.copy(dst[:, s4 * 512:(s4 + 1) * 512], ps[:])
                store[mc] = dst

        v_t = {}

        def v_proj(sc):
            # v[s, m] tile for j-chunk sc: per head h cols h*65..h*65+63 = v,
            # col h*65+64 = 1.0 (softmax denominator column)
            vt = p_v.tile([JC, HPG * (DH + 1)], F32R, tag="v", name=f"v{sc}")
            nc.vector.tensor_copy(
                vt[:].rearrange("p (h e) -> p h e", h=HPG)[:, :, DH:DH + 1].squeeze(2),
                onesb_t[:])
            ps = ps_big.tile([128, 512], F32, tag="proj")
            for dc in range(DC):
                nc.tensor.matmul(
                    ps[:, 0:M],
                    x_t[dc][:, sc * 128:(sc + 1) * 128],
                    w_t[dc][:, 2 * M:3 * M],
                    start=(dc == 0), stop=(dc == DC - 1))
            src = ps[:, 0:M].rearrange("p (h d) -> p h d", h=HPG)
            dst = vt[:].rearrange("p (h e) -> p h e", h=HPG)[:, :, 0:DH]
            nc.vector.tensor_copy(dst, src)
            v_t[sc] = vt

        # ---- attention, scoresT orientation ----
        # Emission is software-pipelined: the normalize/evict of a group
        # (reciprocal -> PE ones-broadcast -> DVE mul) is emitted one group
        # later so the reciprocal never stalls the in-order PE stream.
        # o-projection blocks are emitted as soon as their i-range has all
        # 4 heads normalized.
        mg_t = [p_mg.tile([128, S], F32R, tag="mgT", name=f"mg{i}")
                for i in range(M // 128)]

        def attend(h, ic):
            # j-chunks processed in PAIRS sharing a [128, 2*IC] PSUM tile and
            # a single exp instruction (halves ACT instruction count). Within
            # a sub-block of 4 pairs: all scores first, then all AVs, so the
            # in-order PE stream never waits on ACT latency.
            qk_tile = h // 2
            prow = 64 * (h % 2)
            njc = (ic * IC) // JC + IC // JC  # causal: j chunks 0..njc-1
            at_ps = ps_at.tile([DH + 1, IC], F32, tag="attn")
            pairs = [(p, min(p + 2, njc)) for p in range(0, njc, 2)]
            SUBP = 4
            for p0 in range(0, len(pairs), SUBP):
                blk = pairs[p0:p0 + SUBP]
                prs = []
                for (ja, jb) in blk:
                    sc_ps = ps_sc.tile([128, 2 * IC], F32, tag="scores")
                    pr = p_probs.tile([JC, 2 * IC], F32R, tag="probs")
                    for u, jc in enumerate(range(ja, jb)):
                        nc.tensor.matmul(
                            sc_ps[:, u * IC:(u + 1) * IC],
                            k_t[qk_tile][prow:prow + DH, jc * JC:(jc + 1) * JC],
                            q_t[qk_tile][prow:prow + DH, ic * IC:(ic + 1) * IC],
                            start=True, stop=True)
                    nc.scalar.activation(pr[:], sc_ps[:], EXP, scale=SCALE)
                    for u, jc in enumerate(range(ja, jb)):
                        delta = jc * JC - ic * IC
                        if delta >= 0:
                            # diagonal block: columns >= delta+128 are all-keep,
                            # so multiply only [0, delta+128) -- the mask slice
                            # is all-zero left of the 128-wide triangular strip.
                            w = delta + JC
                            nc.vector.tensor_mul(
                                pr[:, u * IC:u * IC + w],
                                pr[:, u * IC:u * IC + w],
                                mask_t[:, 384 - delta:384 - delta + w])
                    prs.append(pr)
                for (ja, jb), pr in zip(blk, prs):
                    for u, jc in enumerate(range(ja, jb)):
                        nc.tensor.matmul(
                            at_ps[:],
                            v_t[jc][:, h * (DH + 1):(h + 1) * (DH + 1)],
                            pr[:, u * IC:(u + 1) * IC],
                            start=(jc == 0), stop=(jc == njc - 1))
            return at_ps

        def normalize(h, ic, at_ps):
            # rows 0..63 / row 64 (denominator), evicted into mergedT
            qk_tile = h // 2
            prow = 64 * (h % 2)
            den = p_small.tile([1, IC], F32, tag="den")
            nc.vector.tensor_copy(den[:], at_ps[DH:DH + 1, :])
            rc32 = p_small.tile([1, IC], F32, tag="recip32")
            nc.vector.reciprocal_approx_fast(rc32[:], den[:])
            rc = p_small.tile([1, IC], F32R, tag="recip")
            nc.vector.tensor_copy(rc[:], rc32[:])
            bc_ps = ps_big.tile([DH, IC], F32, tag="proj")
            nc.tensor.matmul(bc_ps[:], ones_t[:], rc[:], start=True, stop=True)
            bc_sb = p_small.tile([DH, IC], F32, tag="bcast")
            nc.vector.tensor_copy(bc_sb[:], bc_ps[:])
            nc.vector.tensor_mul(
                mg_t[qk_tile][prow:prow + DH, ic * IC:(ic + 1) * IC],
                at_ps[0:DH, :], bc_sb[:])

        def oproj(sc):
            # out[s, o] = sum_k mergedT[k, s] woT[k, o] for s-chunk sc.
            # The two half-evictions are split across DVE and ACT to keep
            # either engine from becoming the attention-phase bottleneck.
            stg = p_ostg.tile([128, D], F32, tag="ostg")
            for nn in range(2):
                ps = ps_big.tile([128, 512], F32, tag="proj")
                for kc in range(2):
                    nc.tensor.matmul(
                        ps[:],
                        mg_t[kc][:, sc * 128:(sc + 1) * 128],
                        wo_t[kc][:, nn * 512:(nn + 1) * 512],
                        start=(kc == 0), stop=(kc == 1))
                if nn == 0:
                    nc.vector.tensor_copy(stg[:, 0:512], ps[:])
                else:
                    nc.scalar.copy(stg[:, 512:1024], ps[:])
                nc.sync.dma_start(
                    out_d[sc * 128:(sc + 1) * 128, nn * 512:(nn + 1) * 512],
                    stg[:, nn * 512:(nn + 1) * 512])

        # ---- interleaved schedule ----
        # Attention groups (ascending ic) are woven between projection blocks
        # so ACT's exp stream overlaps the PE-dense projection phase, and the
        # normalize/o-proj of a group is emitted one group later so neither
        # the reciprocal chain nor the mergedT eviction gates the in-order PE
        # stream.
        sched = [
            ("qk", 0), ("v", 0, 4),
            ("a", 0, 0), ("a", 1, 0),
            ("qk", 1),
            ("a", 2, 0), ("a", 3, 0),
            ("v", 4, 8),
            ("a", 0, 1), ("a", 1, 1), ("a", 2, 1), ("a", 3, 1),
            ("v", 8, 12),
            ("a", 0, 2), ("a", 1, 2), ("a", 2, 2), ("a", 3, 2),
            ("v", 12, 16),
            ("a", 0, 3), ("a", 1, 3), ("a", 2, 3), ("a", 3, 3),
        ]
        pending = None
        pending_oproj = []
        for item in sched:
            if item[0] == "qk":
                qk_proj(item[1])
                continue
            if item[0] == "v":
                for sc in range(item[1], item[2]):
                    v_proj(sc)
                continue
            _, h, ic = item
            at = attend(h, ic)
            for sc in pending_oproj:
                oproj(sc)
            pending_oproj = []
            if pending is not None:
                normalize(*pending)
                if pending[0] == HPG - 1:  # last head of its ic: mergedT done
                    pending_oproj = list(range(4 * pending[1], 4 * pending[1] + 4))
            pending = (h, ic, at)
        normalize(*pending)
        for sc in pending_oproj + list(range(4 * pending[1], 4 * pending[1] + 4)):
            oproj(sc)


_NC_CACHE = None


def _get_nc():
    global _NC_CACHE
    if _NC_CACHE is None:
        _NC_CACHE = _build_nc()
    return _NC_CACHE


def _causal_mask_tile():
    # BIGMASK[j, c] = 1.0 if j <= c - 384 else 0.0, shape [128, 896].
    # Diagonal block at delta = j_base - i_base uses cols [384-delta, 384-delta+512).
    j = np.arange(JC)[:, None]
    c = np.arange(896)[None, :]
    return (j <= c - 384).astype(np.float32)


def _prepare_in_maps(inputs):
    x = np.asarray(inputs["in_features"], dtype=np.float32)
    wqT = np.ascontiguousarray(np.asarray(inputs["q_proj_weight"], np.float32).T)
    wkT = np.ascontiguousarray(np.asarray(inputs["k_proj_weight"], np.float32).T)
    wvT = np.ascontiguousarray(np.asarray(inputs["v_proj_weight"], np.float32).T)
    woT = np.ascontiguousarray(np.asarray(inputs["o_proj_weight"], np.float32).T)
    xT = [np.ascontiguousarray(x[b].T) for b in range(B)]
    mask = _causal_mask_tile()

    in_maps = []
    for c in range(NCORES):
        b, g = divmod(c, HPG)
        ms = slice(g * M, (g + 1) * M)
        in_maps.append({
            "xT": xT[b],
            "wqkvT": np.ascontiguousarray(
                np.concatenate([wqT[:, ms], wkT[:, ms], wvT[:, ms]], axis=1)),
            "woT": np.ascontiguousarray(woT[ms, :]),
            "mask": mask,
            "ones_a": np.ones((1, 64), np.float32),
            "ones_b": np.ones((JC, HPG), np.float32),
        })
    return in_maps


def kernel(q_proj_weight, k_proj_weight, v_proj_weight, o_proj_weight, in_features):
    in_dtype = np.asarray(in_features).dtype
    in_maps = _prepare_in_maps({
        "q_proj_weight": q_proj_weight,
        "k_proj_weight": k_proj_weight,
        "v_proj_weight": v_proj_weight,
        "o_proj_weight": o_proj_weight,
        "in_features": in_features,
    })
    nc = _get_nc()
    res = bass_utils.run_bass_kernel_spmd(nc, in_maps, core_ids=list(range(NCORES)))
    out = np.zeros((B, S, D), dtype=np.float32)
    for c in range(NCORES):
        out[c // HPG] += res.results[c]["out"]
    return out.astype(in_dtype)


# revision 31
# speedup vs baseline: 1.0270x; 1.0270x over previous
"""Multi-head self-attention (B=2, S=2048, D=1024, H=16, causal) on 8 NeuronCores.

Sharding: core c = 4*b + g handles batch b and heads 4g..4g+3 (batch x
head-group parallel). Per core:
  - q/k projections in transposed layout  qT/kT [dh, s]  (dh on partitions)
  - v projection in natural layout [s, dh] with a fused ones-column per head
    (gives the softmax denominator for free during the AV matmul)
  - causal attention in scoresT [j, i] orientation: PE scores -> ACT exp
    (scale=1/8, no max subtraction; scores ~ N(0,1) so exp never overflows)
    -> DVE causal mask multiply on diagonal blocks -> PE AV accumulate
  - normalization of attnT by the per-query denominator via a PE ones-
    broadcast + DVE multiply during PSUM eviction
  - partial o-projection out_c = merged_c @ Wo[:, cols_c].T
Host sums the 4 partial outputs per batch (the only cross-core reduction).

All matmuls run in float32r (full-rate fp32 on the TRN2 PE).
"""

import numpy as np

import concourse.bass as bass
from concourse import bacc
import concourse.mybir as mybir
import concourse.tile as tile
from concourse import bass_utils

F32 = mybir.dt.float32
F32R = mybir.dt.float32r
EXP = mybir.ActivationFunctionType.Exp

B, S, D = 2, 2048, 1024
H, DH = 16, 64
NCORES = 8
HPG = 4                  # heads per group (per core)
M = HPG * DH             # 256 per-core head dims
DC = D // 128            # 8 contraction chunks for projections
IC = 512                 # i (query) chunk for attention
JC = 128                 # j (key) chunk for attention
SCALE = 1.0 / np.sqrt(DH)


def _build_nc():
    nc = bacc.Bacc("TRN2", target_bir_lowering=False, debug=False)

    xT_d = nc.dram_tensor("xT", [D, S], F32R, kind="ExternalInput").ap()
    wqkv_d = nc.dram_tensor("wqkvT", [D, 3 * M], F32R, kind="ExternalInput").ap()
    woT_d = nc.dram_tensor("woT", [M, D], F32R, kind="ExternalInput").ap()
    mask_d = nc.dram_tensor("mask", [JC, 896], F32, kind="ExternalInput").ap()
    onesa_d = nc.dram_tensor("ones_a", [1, 64], F32R, kind="ExternalInput").ap()
    onesb_d = nc.dram_tensor("ones_b", [JC, HPG], F32R, kind="ExternalInput").ap()
    out_d = nc.dram_tensor("out", [S, D], F32, kind="ExternalOutput").ap()

    with tile.TileContext(nc) as tc:
        _body(tc, xT_d, wqkv_d, woT_d, mask_d, onesa_d, onesb_d, out_d)
    nc.compile()
    return nc


def _body(tc, xT_d, wqkv_d, woT_d, mask_d, onesa_d, onesb_d, out_d):
    nc = tc.nc
    from contextlib import ExitStack
    ctx = ExitStack()
    with ctx:
        p_x = ctx.enter_context(tc.tile_pool(name="x", bufs=DC))
        p_w = ctx.enter_context(tc.tile_pool(name="w", bufs=DC))
        p_wo = ctx.enter_context(tc.tile_pool(name="wo", bufs=2))
        p_qk = ctx.enter_context(tc.tile_pool(name="qk", bufs=2))
        p_v = ctx.enter_context(tc.tile_pool(name="v", bufs=S // JC))
        p_mg = ctx.enter_context(tc.tile_pool(name="mg", bufs=2))
        p_probs = ctx.enter_context(tc.tile_pool(name="probs", bufs=5))
        p_small = ctx.enter_context(tc.tile_pool(name="small", bufs=1))
        p_mask = ctx.enter_context(tc.tile_pool(name="mask", bufs=1))
        p_ostg = ctx.enter_context(tc.tile_pool(name="ostg", bufs=2))
        p_ones = ctx.enter_context(tc.tile_pool(name="ones", bufs=1))

        ps_big = ctx.enter_context(tc.tile_pool(name="psb", bufs=2, space="PSUM"))
        ps_sc = ctx.enter_context(tc.tile_pool(name="pss", bufs=2, space="PSUM"))
        ps_at = ctx.enter_context(tc.tile_pool(name="psa", bufs=2, space="PSUM"))

        # ---- input loads: combined qkv weight tile + x tile interleaved in
        # the order the projection matmuls consume them (dc ascending), so
        # the first matmul can start ~5us in instead of after the full load.
        w_t, x_t = [], []
        for dc in range(DC):
            wt = p_w.tile([128, 3 * M], F32R, tag="w")
            nc.sync.dma_start(wt[:], wqkv_d[dc * 128:(dc + 1) * 128, :])
            w_t.append(wt)
            xt = p_x.tile([128, S], F32R, tag="x")
            nc.sync.dma_start(xt[:], xT_d[dc * 128:(dc + 1) * 128, :])
            x_t.append(xt)
        wo_t = []
        for kc in range(2):
            t = p_wo.tile([128, D], F32R, tag="wo")
            nc.sync.dma_start(t[:], woT_d[kc * 128:(kc + 1) * 128, :])
            wo_t.append(t)
        mask_t = p_mask.tile([JC, 896], F32, tag="mask")
        nc.sync.dma_start(mask_t[:], mask_d[:])
        ones_t = p_ones.tile([1, 64], F32R, tag="ones")
        nc.sync.dma_start(ones_t[:], onesa_d[:])
        onesb_t = p_ones.tile([JC, HPG], F32R, tag="onesb")
        nc.sync.dma_start(onesb_t[:], onesb_d[:])

        # ---- projection building blocks ----
        q_t, k_t = {}, {}

        def qk_proj(mc):
            # qT/kT [m, s] = sum_d WT[d, m] xT[d, s], m-chunk mc
            for woff, store, tg in ((0, q_t, "qT"), (M, k_t, "kT")):
                dst = p_qk.tile([128, S], F32R, tag=tg, name=f"{tg}{mc}")
                for s4 in range(S // 512):
                    ps = ps_big.tile([128, 512], F32, tag="proj")
                    for dc in range(DC):
                        nc.tensor.matmul(
                            ps[:],
                            w_t[dc][:, woff + mc * 128:woff + (mc + 1) * 128],
                            x_t[dc][:, s4 * 512:(s4 + 1) * 512],
                            start=(dc == 0), stop=(dc == DC - 1))
                    nc.scalar.copy(dst[:, s4 * 512:(s4 + 1) * 512], ps[:])
                store[mc] = dst

        v_t = {}

        def v_proj(sc):
            # v[s, m] tile for j-chunk sc: per head h cols h*65..h*65+63 = v,
            # col h*65+64 = 1.0 (softmax denominator column)
            vt = p_v.tile([JC, HPG * (DH + 1)], F32R, tag="v", name=f"v{sc}")
            nc.vector.tensor_copy(
                vt[:].rearrange("p (h e) -> p h e", h=HPG)[:, :, DH:DH + 1].squeeze(2),
                onesb_t[:])
            ps = ps_big.tile([128, 512], F32, tag="proj")
            for dc in range(DC):
                nc.tensor.matmul(
                    ps[:, 0:M],
                    x_t[dc][:, sc * 128:(sc + 1) * 128],
                    w_t[dc][:, 2 * M:3 * M],
                    start=(dc == 0), stop=(dc == DC - 1))
            src = ps[:, 0:M].rearrange("p (h d) -> p h d", h=HPG)
            dst = vt[:].rearrange("p (h e) -> p h e", h=HPG)[:, :, 0:DH]
            nc.vector.tensor_copy(dst, src)
            v_t[sc] = vt

        # ---- attention, scoresT orientation ----
        # Emission is software-pipelined: the normalize/evict of a group
        # (reciprocal -> PE ones-broadcast -> DVE mul) is emitted one group
        # later so the reciprocal never stalls the in-order PE stream.
        # o-projection blocks are emitted as soon as their i-range has all
        # 4 heads normalized.
        mg_t = [p_mg.tile([128, S], F32R, tag="mgT", name=f"mg{i}")
                for i in range(M // 128)]

        def attend(h, ic):
            # j-chunks processed in PAIRS sharing a [128, 2*IC] PSUM tile and
            # a single exp instruction (halves ACT instruction count). Within
            # a sub-block of 4 pairs: all scores first, then all AVs, so the
            # in-order PE stream never waits on ACT latency.
            qk_tile = h // 2
            prow = 64 * (h % 2)
            njc = (ic * IC) // JC + IC // JC  # causal: j chunks 0..njc-1
            at_ps = ps_at.tile([DH + 1, IC], F32, tag="attn")
            pairs = [(p, min(p + 2, njc)) for p in range(0, njc, 2)]
            SUBP = 4
            for p0 in range(0, len(pairs), SUBP):
                blk = pairs[p0:p0 + SUBP]
                prs = []
                for (ja, jb) in blk:
                    sc_ps = ps_sc.tile([128, 2 * IC], F32, tag="scores")
                    pr = p_probs.tile([JC, 2 * IC], F32R, tag="probs")
                    for u, jc in enumerate(range(ja, jb)):
                        nc.tensor.matmul(
                            sc_ps[:, u * IC:(u + 1) * IC],
                            k_t[qk_tile][prow:prow + DH, jc * JC:(jc + 1) * JC],
                            q_t[qk_tile][prow:prow + DH, ic * IC:(ic + 1) * IC],
                            start=True, stop=True)
                    nc.scalar.activation(pr[:], sc_ps[:], EXP, scale=SCALE)
                    for u, jc in enumerate(range(ja, jb)):
                        delta = jc * JC - ic * IC
                        if delta >= 0:
                            # diagonal block: columns >= delta+128 are all-keep,
                            # so multiply only [0, delta+128) -- the mask slice
                            # is all-zero left of the 128-wide triangular strip.
                            w = delta + JC
                            nc.vector.tensor_mul(
                                pr[:, u * IC:u * IC + w],
                                pr[:, u * IC:u * IC + w],
                                mask_t[:, 384 - delta:384 - delta + w])
                    prs.append(pr)
                for (ja, jb), pr in zip(blk, prs):
                    for u, jc in enumerate(range(ja, jb)):
                        nc.tensor.matmul(
                            at_ps[:],
                            v_t[jc][:, h * (DH + 1):(h + 1) * (DH + 1)],
                            pr[:, u * IC:(u + 1) * IC],
                            start=(jc == 0), stop=(jc == njc - 1))
            return at_ps

        def normalize(h, ic, at_ps):
            # rows 0..63 / row 64 (denominator), evicted into mergedT
            qk_tile = h // 2
            prow = 64 * (h % 2)
            den = p_small.tile([1, IC], F32, tag="den")
            nc.vector.tensor_copy(den[:], at_ps[DH:DH + 1, :])
            rc32 = p_small.tile([1, IC], F32, tag="recip32")
            nc.vector.reciprocal_approx_fast(rc32[:], den[:])
            rc = p_small.tile([1, IC], F32R, tag="recip")
            nc.vector.tensor_copy(rc[:], rc32[:])
            bc_ps = ps_big.tile([DH, IC], F32, tag="proj")
            nc.tensor.matmul(bc_ps[:], ones_t[:], rc[:], start=True, stop=True)
            bc_sb = p_small.tile([DH, IC], F32, tag="bcast")
            nc.vector.tensor_copy(bc_sb[:], bc_ps[:])
            nc.vector.tensor_mul(
                mg_t[qk_tile][prow:prow + DH, ic * IC:(ic + 1) * IC],
                at_ps[0:DH, :], bc_sb[:])

        def oproj(sc):
            # out[s, o] = sum_k mergedT[k, s] woT[k, o] for s-chunk sc.
            # The two half-evictions are split across DVE and ACT to keep
            # either engine from becoming the attention-phase bottleneck.
            stg = p_ostg.tile([128, D], F32, tag="ostg")
            for nn in range(2):
                ps = ps_big.tile([128, 512], F32, tag="proj")
                for kc in range(2):
                    nc.tensor.matmul(
                        ps[:],
                        mg_t[kc][:, sc * 128:(sc + 1) * 128],
                        wo_t[kc][:, nn * 512:(nn + 1) * 512],
                        start=(kc == 0), stop=(kc == 1))
                if nn == 0:
                    nc.vector.tensor_copy(stg[:, 0:512], ps[:])
                else:
                    nc.scalar.copy(stg[:, 512:1024], ps[:])
            nc.sync.dma_start(out_d[sc * 128:(sc + 1) * 128, :], stg[:])

        # ---- interleaved schedule ----
        # Attention groups (ascending ic) are woven between projection blocks
        # so ACT's exp stream overlaps the PE-dense projection phase, and the
        # normalize/o-proj of a group is emitted one group later so neither
        # the reciprocal chain nor the mergedT eviction gates the in-order PE
        # stream.
        sched = [
            ("qk", 0), ("v", 0, 4),
            ("a", 0, 0), ("a", 1, 0),
            ("qk", 1),
            ("a", 2, 0), ("a", 3, 0),
            ("v", 4, 8),
            ("a", 0, 1), ("a", 1, 1), ("a", 2, 1), ("a", 3, 1),
            ("v", 8, 12),
            ("a", 0, 2), ("a", 1, 2), ("a", 2, 2), ("a", 3, 2),
            ("v", 12, 16),
            ("a", 0, 3), ("a", 1, 3), ("a", 2, 3), ("a", 3, 3),
        ]
        pending = None
        pending_oproj = []
        for item in sched:
            if item[0] == "qk":
                qk_proj(item[1])
                continue
            if item[0] == "v":
                for sc in range(item[1], item[2]):
                    v_proj(sc)
                continue
            _, h, ic = item
            at = attend(h, ic)
            for sc in pending_oproj:
                oproj(sc)
            pending_oproj = []
            if pending is not None:
                normalize(*pending)
                if pending[0] == HPG - 1:  # last head of its ic: mergedT done
                    pending_oproj = list(range(4 * pending[1], 4 * pending[1] + 4))
            pending = (h, ic, at)
        normalize(*pending)
        for sc in pending_oproj + list(range(4 * pending[1], 4 * pending[1] + 4)):
            oproj(sc)


_NC_CACHE = None


def _get_nc():
    global _NC_CACHE
    if _NC_CACHE is None:
        _NC_CACHE = _build_nc()
    return _NC_CACHE


def _causal_mask_tile():
    # BIGMASK[j, c] = 1.0 if j <= c - 384 else 0.0, shape [128, 896].
    # Diagonal block at delta = j_base - i_base uses cols [384-delta, 384-delta+512).
    j = np.arange(JC)[:, None]
    c = np.arange(896)[None, :]
    return (j <= c - 384).astype(np.float32)


def _prepare_in_maps(inputs):
    x = np.asarray(inputs["in_features"], dtype=np.float32)
    wqT = np.ascontiguousarray(np.asarray(inputs["q_proj_weight"], np.float32).T)
    wkT = np.ascontiguousarray(np.asarray(inputs["k_proj_weight"], np.float32).T)
    wvT = np.ascontiguousarray(np.asarray(inputs["v_proj_weight"], np.float32).T)
    woT = np.ascontiguousarray(np.asarray(inputs["o_proj_weight"], np.float32).T)
    xT = [np.ascontiguousarray(x[b].T) for b in range(B)]
    mask = _causal_mask_tile()

    in_maps = []
    for c in range(NCORES):
        b, g = divmod(c, HPG)
        ms = slice(g * M, (g + 1) * M)
        in_maps.append({
            "xT": xT[b],
            "wqkvT": np.ascontiguousarray(
                np.concatenate([wqT[:, ms], wkT[:, ms], wvT[:, ms]], axis=1)),
            "woT": np.ascontiguousarray(woT[ms, :]),
            "mask": mask,
            "ones_a": np.ones((1, 64), np.float32),
            "ones_b": np.ones((JC, HPG), np.float32),
        })
    return in_maps


def kernel(q_proj_weight, k_proj_weight, v_proj_weight, o_proj_weight, in_features):
    in_dtype = np.asarray(in_features).dtype
    in_maps = _prepare_in_maps({
        "q_proj_weight": q_proj_weight,
        "k_proj_weight": k_proj_weight,
        "v_proj_weight": v_proj_weight,
        "o_proj_weight": o_proj_weight,
        "in_features": in_features,
    })
    nc = _get_nc()
    res = bass_utils.run_bass_kernel_spmd(nc, in_maps, core_ids=list(range(NCORES)))
    out = np.zeros((B, S, D), dtype=np.float32)
    for c in range(NCORES):
        out[c // HPG] += res.results[c]["out"]
    return out.astype(in_dtype)
